# revision 8
# baseline (speedup 1.0000x reference)
"""Trainium2 Bass kernel for nn_Aggregation0 (scatter_memory).

8 cores = 4 frames x 2 image-halves (SPMD, one program). The host pre-sorts
patch rows into destination order per core; patches are stored j-major
(j, i, c) so the fold matmuls stream contiguous 21-element runs.

Device pipeline, software-pipelined as fold(k+1) between merge(k) and the
rest of unfold(k) so PE folds while DVE/ACT stitch and evacuate. Tops are
blocked [9, 36, 36, 33, 12, 5]: a small head block starts compute early
behind the DMA stream, small tail blocks shorten the pipeline drain.
  fold: per (chunk, j, g mod 7) bf16 matmuls vs shifted-identity weights;
    the g-residue split keeps one matmul's out runs non-overlapping while
    the fold overlap accumulates across matmuls in PSUM (has_written).
  merge: DVE stitches the 6-row block overlap, scaled by 1/cnt(col)
  vb: PE transpose to (row,ch)-major; ACT evacuates to SBUF as bf16 with a
    fused per-partition 1/cnt(row) scale (completing the 1/w normalize)
  pj: per (j, left-chunk) bf16 PE transpose back to column-partition layout
  assembly: DVE/ACT strided copies expand rows into patches (bf16)
  -> sequential bf16 store (host inverse-maps).
DMA: input stream + small consts on the SP HWDGE ring, outputs + identw on
the ACT ring so the two streams don't FIFO-serialize.
"""
import sys
if '/opt/trn_rl_repo' not in sys.path:
    sys.path.insert(0, '/opt/trn_rl_repo')
import numpy as np

import concourse.bacc as bacc
import concourse.bass as bass
import concourse.mybir as mybir
import concourse.tile as tile
from concourse.bass_utils import run_bass_kernel_spmd

T, HP, WP = 4, 256, 256
PS = 7
NPOS = 250
P = NPOS * NPOS
BT = [9, 36, 36, 33, 12, 5]  # tops per block
NB = len(BT)
SB = [sum(BT[:k]) for k in range(NB)]     # block start top
RW = 42                      # rows per vt/vtu tile (max B + 6)
RC = 3 * RW                  # 126 (rl, c) columns
GW = 131
NREAL = GW * NPOS            # 32750
F32 = mybir.dt.float32
BF16 = mybir.dt.bfloat16
D = 147
# out-tops per block k: contiguous partition of [0, 130]
OT = [(max(0, SB[k] - 6),
       (SB[k + 1] - 7) if k + 1 < NB else 130) for k in range(NB)]
SIN = sum(BT) * 2            # 262 input slots of [128, 147]
SOUT = sum(t[1] - t[0] + 1 for t in OT) * 2   # 262 output slots
IN_W = SIN * D
OUT_W = SOUT * D

# device patch element order is (j, i, ch); COLPERM maps back to (ch, i, j)
COLPERM = np.zeros(147, np.int64)
PELEM = np.zeros(147, np.int64)
for _c in range(3):
    for _i in range(7):
        for _j in range(7):
            COLPERM[_c * 49 + _i * 7 + _j] = _j * 21 + _i * 3 + _c
            PELEM[_j * 21 + _i * 3 + _c] = _c * 49 + _i * 7 + _j


def _cntf(z):
    z = np.asarray(z, np.float64)
    return np.minimum(6, z) - np.maximum(0, z - 249) + 1


def _host_prep_core(x, nlInds, c):
    f, h = c >> 1, c & 1
    g0 = 0 if h == 0 else 119
    o_lo, o_hi = (0, 124) if h == 0 else (6, 130)
    inds = nlInds[f, :, 0]
    top = inds[:, 1].astype(np.int64)
    left = inds[:, 2].astype(np.int64)
    invperm = np.empty(P, np.int64)
    invperm[top * NPOS + left] = np.arange(P)
    sel = np.nonzero((top >= g0) & (top <= g0 + 130))[0]
    rank = np.full(P, -1, np.int64)
    rank[sel] = np.arange(NREAL)
    ar128 = np.arange(128)
    DM = -1
    gidx = np.zeros((SIN, 128), np.int64)
    s = 0
    for k in range(NB):
        for g in range(BT[k]):
            gt_ = g0 + SB[k] + g
            for ci, base in ((0, 0), (1, 122)):
                gidx[s] = rank[invperm[gt_ * NPOS + base + ar128]]
                s += 1
    sidx = np.full((SOUT, 128), DM, np.int64)
    s = 0
    for k in range(NB):
        t_lo, t_hi = OT[k]
        for lt in range(t_lo, t_hi + 1):
            gt_ = g0 + lt
            for ci, base in ((0, 0), (1, 122)):
                if o_lo <= lt <= o_hi:
                    col = rank[invperm[gt_ * NPOS + base + ar128]]
                    sidx[s] = col
                    if ci == 1:
                        sidx[s, :6] = DM
                s += 1
    # x plane: (128, SIN*147) dest-ordered slots, (j, i, c) element order, bf16
    import ml_dtypes
    xp = np.concatenate([x[f, sel, 0], np.zeros((1, D), np.float32)], axis=0)
    xs = xp[gidx.reshape(-1)][:, PELEM].reshape(SIN, 128, D) \
        .transpose(1, 0, 2).reshape(128, IN_W)
    x_bf = np.ascontiguousarray(xs.astype(ml_dtypes.bfloat16))
    # normalization, factorized: colinv[p, chunk] = 1/cnt(col),
    # rowinv[rl*3+c, k] = 1/cnt(row) (0 for rows outside the image)
    colinv = np.zeros((128, 2), np.float32)
    for chunk in range(2):
        colinv[:, chunk] = 1.0 / _cntf(chunk * 128 + np.arange(128))
    rowinv = np.zeros((128, NB), np.float32)
    for k in range(NB):
        for rl in range(RW):
            gr = g0 + SB[k] - 6 + rl
            lr = SB[k] - 6 + rl
            if 0 <= gr <= 255 and 0 <= lr <= 136:
                rowinv[rl * 3:rl * 3 + 3, k] = 1.0 / _cntf(gr)
    return dict(x_bf=x_bf, colinv=colinv, rowinv=rowinv,
                f=f, sel=sel, sidx=sidx)


def _identw():
    w = np.zeros((128, 262), np.float32)
    w[np.arange(128), np.arange(128) + 128] = 1.0
    return w


def _ap(base, off, dims):
    return bass.AP(base.tensor, base.offset + off,
                   [list(base.ap[0])] + [list(d) for d in dims])


def build_nc():
    nc = bacc.Bacc("TRN2", target_bir_lowering=False, debug=False, num_devices=8)
    xb_d = nc.declare_dram_parameter("x_bf", [128, IN_W], BF16, isOutput=False)
    ib_d = nc.declare_dram_parameter("identb", [128, 262], BF16, isOutput=False)
    ci_d = nc.declare_dram_parameter("colinv", [128, 2], F32, isOutput=False)
    ri_d = nc.declare_dram_parameter("rowinv", [128, NB], F32, isOutput=False)
    id_d = nc.declare_dram_parameter("identw", [128, 262], F32, isOutput=False)
    y_d = nc.declare_dram_parameter("y_core", [128, OUT_W], BF16, isOutput=True)

    with tile.TileContext(nc) as tc:
        with tc.tile_pool(name="const", bufs=1) as cpool, \
             tc.tile_pool(name="gp", bufs=3) as gpool, \
             tc.tile_pool(name="vtp", bufs=2, space="PSUM") as vtps, \
             tc.tile_pool(name="vtu", bufs=2) as vtup, \
             tc.tile_pool(name="vbp", bufs=1, space="PSUM") as vbp, \
             tc.tile_pool(name="vsb", bufs=2) as vsbp, \
             tc.tile_pool(name="pjp", bufs=3, space="PSUM") as pjp, \
             tc.tile_pool(name="stg", bufs=2) as stgp:
            # identb + tiny norm tables lead the SP ring (identb gates the
            # first fold); identw (first needed by vb) and the outputs ride
            # the ACT ring so in/out streams don't serialize.
            identb = cpool.tile([128, 262], BF16)
            nc.sync.dma_start(out=identb[:], in_=ib_d[:])
            cvt = cpool.tile([128, 2], F32)
            nc.sync.dma_start(out=cvt[:], in_=ci_d[:])
            rvt = cpool.tile([128, NB], F32)
            nc.sync.dma_start(out=rvt[:], in_=ri_d[:])
            identw = cpool.tile([128, 262], F32)
            nc.scalar.dma_start(out=identw[:], in_=id_d[:])

            vt_hist = [None] * NB
            vtu_hist = [None] * NB
            in_off = [0]
            out_off = [0]

            def fold_block(k):
                G = BT[k]
                w = G * 2 * D
                gth = gpool.tile([128, w], BF16, tag="gth", name=f"gth{k}")
                nc.sync.dma_start(out=gth[:],
                                  in_=xb_d[:, in_off[0]: in_off[0] + w])
                in_off[0] += w
                r_last = min(6, G - 1)
                vts = []
                for chunk in range(2):
                    vt = vtps.tile([128, RC], F32, tag=f"vt{chunk}",
                                   name=f"vt{chunk}_{k}")
                    # g is split by residue mod 7 so one matmul's out runs
                    # (21m + 3i + c) never self-overlap; the fold overlap
                    # accumulates across matmuls via PSUM has_written.
                    for j in range(7):
                        d = j if chunk == 0 else j - 6
                        for r in range(7):
                            Gr = len(range(r, G, 7))
                            if Gr == 0:
                                continue
                            rhs = _ap(gth[:],
                                      chunk * D + r * 2 * D + j * 21,
                                      [(14 * D, Gr), (3, 7), (1, 3)])
                            out = _ap(vt[:], 3 * r,
                                      [(21, Gr), (3, 7), (1, 3)])
                            nc.tensor.matmul(
                                out, lhsT=identb[:, 128 - d:256 - d],
                                rhs=rhs,
                                start=(j == 0 and r == 0),
                                stop=(j == 6 and r == r_last))
                    vts.append(vt)
                vt_hist[k] = vts
                if k >= 2:
                    vt_hist[k - 2] = None

            def merge_block(k):
                G = BT[k]
                n1 = min(G, 30)              # rows merged from vt_k
                vtus = []
                for chunk in range(2):
                    vt = vt_hist[k][chunk]
                    cv = cvt[:, chunk:chunk + 1]
                    vtu = vtup.tile([128, RC], F32, tag=f"vtu{chunk}",
                                    name=f"vtu{chunk}_{k}")
                    nc.vector.tensor_scalar_mul(
                        out=vtu[:, 36:36 + 3 * n1],
                        in0=vt[:, 18:18 + 3 * n1], scalar1=cv)
                    if k > 0:
                        Bp = BT[k - 1]
                        vtp = vt_hist[k - 1][chunk]
                        nc.vector.tensor_scalar_mul(
                            out=vtu[:, 0:18],
                            in0=vtp[:, 3 * (Bp - 6):3 * (Bp - 6) + 18],
                            scalar1=cv)
                        nc.vector.tensor_copy(
                            out=vtu[:, 18:36],
                            in_=vtp[:, 3 * Bp:3 * Bp + 18])
                        nc.vector.tensor_tensor(
                            out=vtu[:, 18:36], in0=vtu[:, 18:36],
                            in1=vt[:, 0:18], op=mybir.AluOpType.add)
                        nc.vector.tensor_scalar_mul(
                            out=vtu[:, 18:36], in0=vtu[:, 18:36], scalar1=cv)
                    else:
                        nc.vector.tensor_scalar_mul(
                            out=vtu[:, 18:36], in0=vt[:, 0:18], scalar1=cv)
                    vtus.append(vtu)
                vtu_hist[k] = vtus

            def unfold_rest(k):
                vtus = vtu_hist[k]
                vtu_hist[k] = None
                vb = vbp.tile([RC, 256], F32, tag="vb", name=f"vb{k}")
                for chunk in range(2):
                    nc.tensor.matmul(
                        vb[:, chunk * 128:(chunk + 1) * 128],
                        lhsT=vtus[chunk][:, 0:RC], rhs=identw[:, 128:256],
                        is_transpose=True,
                        start=(chunk == 0), stop=(chunk == 1))
                vsb = vsbp.tile([RC, 256], BF16, tag="vsb", name=f"vsb{k}")
                nc.scalar.mul(vsb[:], vb[:], rvt[0:RC, k:k + 1])
                t_lo, t_hi = OT[k]
                nt = t_hi - t_lo + 1
                goff = t_lo - (SB[k] - 6)
                w = nt * 2 * D
                stg = stgp.tile([128, w], BF16, tag="stg", name=f"stg{k}")
                cpy = 0
                for j in range(7):
                    for ci, base in ((0, 0), (1, 122)):
                        pj = pjp.tile([128, RC], BF16, tag="pj",
                                      name=f"pj{k}_{j}_{ci}")
                        nc.tensor.matmul(
                            pj[:], lhsT=vsb[:, base + j: base + j + 128],
                            rhs=identb[0:RC, 128:128 + RC],
                            is_transpose=True, start=True, stop=True)
                        src = _ap(pj[:], goff * 3, [(3, nt), (3, 7), (1, 3)])
                        dst = _ap(stg[:], ci * D + j * 21,
                                  [(2 * D, nt), (3, 7), (1, 3)])
                        if cpy % 2 == 0:
                            nc.scalar.copy(out=dst, in_=src)
                        else:
                            nc.vector.tensor_copy(out=dst, in_=src)
                        cpy += 1
                nc.scalar.dma_start(out=y_d[:, out_off[0]: out_off[0] + w],
                                    in_=stg[:])
                out_off[0] += w

            fold_block(0)
            for k in range(NB):
                merge_block(k)
                if k + 1 < NB:
                    fold_block(k + 1)
                unfold_rest(k)

    nc.compile()
    return nc


_NC_CACHE = [None]


def _build_in_maps(x, nlInds):
    cores = [_host_prep_core(x, nlInds, c) for c in range(8)]
    idw = _identw()
    import ml_dtypes
    idb = idw.astype(ml_dtypes.bfloat16)
    in_maps = [dict(x_bf=cr["x_bf"], colinv=cr["colinv"], rowinv=cr["rowinv"],
                    identw=idw, identb=idb) for cr in cores]
    return cores, in_maps


def kernel(x, nlDists, nlInds, pixels_h, pixels_w):
    x = np.ascontiguousarray(np.asarray(x, dtype=np.float32))
    nlInds = np.asarray(nlInds)
    if _NC_CACHE[0] is None:
        _NC_CACHE[0] = build_nc()
    nc = _NC_CACHE[0]
    cores, in_maps = _build_in_maps(x, nlInds)
    res = run_bass_kernel_spmd(nc, in_maps, list(range(8)))
    out = np.zeros((T, P, 1, 147), np.float32)
    for c in range(8):
        cr = cores[c]
        y = np.asarray(res.results[c]["y_core"]).astype(np.float32)
        ys = y.reshape(128, SOUT, D).transpose(1, 0, 2).reshape(-1, D)
        sidx = cr["sidx"].reshape(-1)
        valid = sidx >= 0
        out[cr["f"], cr["sel"][sidx[valid]], 0] = ys[valid][:, COLPERM]
    return out


# revision 9
# speedup vs baseline: 1.0836x; 1.0836x over previous
"""Trainium2 Bass kernel for nn_Aggregation0 (scatter_memory).

8 cores = 4 frames x 2 image-halves (SPMD, one program). The host pre-sorts
patch rows into destination order per core; patches are stored j-major
(j, i, c) so the fold matmuls stream contiguous 21-element runs.

Device pipeline, software-pipelined as fold(k+1) between merge(k) and the
rest of unfold(k) so PE folds while DVE/ACT stitch and evacuate. Tops are
blocked [9, 36, 36, 33, 12, 5]: a small head block starts compute early
behind the DMA stream, small tail blocks shorten the pipeline drain.
  fold: per (chunk, j, g mod 7) bf16 matmuls vs shifted-identity weights;
    the g-residue split keeps one matmul's out runs non-overlapping while
    the fold overlap accumulates across matmuls in PSUM (has_written).
  merge: DVE stitches the 6-row block overlap, scaled by 1/cnt(col)
  vb: PE transpose to (row,ch)-major; ACT evacuates to SBUF as bf16 with a
    fused per-partition 1/cnt(row) scale (completing the 1/w normalize)
  pj: per (j, left-chunk) bf16 PE transpose back to column-partition layout
  assembly: DVE/ACT strided copies expand rows into patches (bf16)
  -> sequential bf16 store (host inverse-maps).
DMA: input stream + small consts on the SP HWDGE ring, outputs + identw on
the ACT ring so the two streams don't FIFO-serialize.
"""
import sys
if '/opt/trn_rl_repo' not in sys.path:
    sys.path.insert(0, '/opt/trn_rl_repo')
import numpy as np

import concourse.bacc as bacc
import concourse.bass as bass
import concourse.mybir as mybir
import concourse.tile as tile
from concourse.bass_utils import run_bass_kernel_spmd

T, HP, WP = 4, 256, 256
PS = 7
NPOS = 250
P = NPOS * NPOS
BT = [36, 36, 36, 23]        # tops per block
NB = len(BT)
SB = [sum(BT[:k]) for k in range(NB)]     # block start top
RW = 42                      # rows per vt/vtu tile (max B + 6)
RC = 3 * RW                  # 126 (rl, c) columns
GW = 131
NREAL = GW * NPOS            # 32750
F32 = mybir.dt.float32
BF16 = mybir.dt.bfloat16
D = 147
# out-tops per block k: contiguous partition of [0, 130]
OT = [(max(0, SB[k] - 6),
       (SB[k + 1] - 7) if k + 1 < NB else 130) for k in range(NB)]
SIN = sum(BT) * 2            # 262 input slots of [128, 147]
SOUT = sum(t[1] - t[0] + 1 for t in OT) * 2   # 262 output slots
IN_W = SIN * D
OUT_W = SOUT * D

# device patch element order is (j, i, ch); COLPERM maps back to (ch, i, j)
COLPERM = np.zeros(147, np.int64)
PELEM = np.zeros(147, np.int64)
for _c in range(3):
    for _i in range(7):
        for _j in range(7):
            COLPERM[_c * 49 + _i * 7 + _j] = _j * 21 + _i * 3 + _c
            PELEM[_j * 21 + _i * 3 + _c] = _c * 49 + _i * 7 + _j


def _cntf(z):
    z = np.asarray(z, np.float64)
    return np.minimum(6, z) - np.maximum(0, z - 249) + 1


def _host_prep_core(x, nlInds, c):
    f, h = c >> 1, c & 1
    g0 = 0 if h == 0 else 119
    o_lo, o_hi = (0, 124) if h == 0 else (6, 130)
    inds = nlInds[f, :, 0]
    top = inds[:, 1].astype(np.int64)
    left = inds[:, 2].astype(np.int64)
    invperm = np.empty(P, np.int64)
    invperm[top * NPOS + left] = np.arange(P)
    sel = np.nonzero((top >= g0) & (top <= g0 + 130))[0]
    rank = np.full(P, -1, np.int64)
    rank[sel] = np.arange(NREAL)
    ar128 = np.arange(128)
    DM = -1
    gidx = np.zeros((SIN, 128), np.int64)
    s = 0
    for k in range(NB):
        for g in range(BT[k]):
            gt_ = g0 + SB[k] + g
            for ci, base in ((0, 0), (1, 122)):
                gidx[s] = rank[invperm[gt_ * NPOS + base + ar128]]
                s += 1
    sidx = np.full((SOUT, 128), DM, np.int64)
    s = 0
    for k in range(NB):
        t_lo, t_hi = OT[k]
        for lt in range(t_lo, t_hi + 1):
            gt_ = g0 + lt
            for ci, base in ((0, 0), (1, 122)):
                if o_lo <= lt <= o_hi:
                    col = rank[invperm[gt_ * NPOS + base + ar128]]
                    sidx[s] = col
                    if ci == 1:
                        sidx[s, :6] = DM
                s += 1
    # x plane: (128, SIN*147) dest-ordered slots, (j, i, c) element order, bf16
    import ml_dtypes
    xp = np.concatenate([x[f, sel, 0], np.zeros((1, D), np.float32)], axis=0)
    xs = xp[gidx.reshape(-1)][:, PELEM].reshape(SIN, 128, D) \
        .transpose(1, 0, 2).reshape(128, IN_W)
    x_bf = np.ascontiguousarray(xs.astype(ml_dtypes.bfloat16))
    # normalization, factorized: colinv[p, chunk] = 1/cnt(col),
    # rowinv[rl*3+c, k] = 1/cnt(row) (0 for rows outside the image)
    colinv = np.zeros((128, 2), np.float32)
    for chunk in range(2):
        colinv[:, chunk] = 1.0 / _cntf(chunk * 128 + np.arange(128))
    rowinv = np.zeros((128, NB), np.float32)
    for k in range(NB):
        for rl in range(RW):
            gr = g0 + SB[k] - 6 + rl
            lr = SB[k] - 6 + rl
            if 0 <= gr <= 255 and 0 <= lr <= 136:
                rowinv[rl * 3:rl * 3 + 3, k] = 1.0 / _cntf(gr)
    return dict(x_bf=x_bf, colinv=colinv, rowinv=rowinv,
                f=f, sel=sel, sidx=sidx)


def _identw():
    w = np.zeros((128, 262), np.float32)
    w[np.arange(128), np.arange(128) + 128] = 1.0
    return w


def _ap(base, off, dims):
    return bass.AP(base.tensor, base.offset + off,
                   [list(base.ap[0])] + [list(d) for d in dims])


def build_nc():
    nc = bacc.Bacc("TRN2", target_bir_lowering=False, debug=False, num_devices=8)
    xb_d = nc.declare_dram_parameter("x_bf", [128, IN_W], BF16, isOutput=False)
    ib_d = nc.declare_dram_parameter("identb", [128, 262], BF16, isOutput=False)
    ci_d = nc.declare_dram_parameter("colinv", [128, 2], F32, isOutput=False)
    ri_d = nc.declare_dram_parameter("rowinv", [128, NB], F32, isOutput=False)
    id_d = nc.declare_dram_parameter("identw", [128, 262], F32, isOutput=False)
    y_d = nc.declare_dram_parameter("y_core", [128, OUT_W], BF16, isOutput=True)

    with tile.TileContext(nc) as tc:
        with tc.tile_pool(name="const", bufs=1) as cpool, \
             tc.tile_pool(name="gp", bufs=3) as gpool, \
             tc.tile_pool(name="vtp", bufs=2, space="PSUM") as vtps, \
             tc.tile_pool(name="vtu", bufs=2) as vtup, \
             tc.tile_pool(name="vbp", bufs=1, space="PSUM") as vbp, \
             tc.tile_pool(name="vsb", bufs=2) as vsbp, \
             tc.tile_pool(name="pjp", bufs=3, space="PSUM") as pjp, \
             tc.tile_pool(name="stg", bufs=2) as stgp:
            # identb + tiny norm tables lead the SP ring (identb gates the
            # first fold); identw (first needed by vb) and the outputs ride
            # the ACT ring so in/out streams don't serialize.
            identb = cpool.tile([128, 262], BF16)
            nc.sync.dma_start(out=identb[:], in_=ib_d[:])
            cvt = cpool.tile([128, 2], F32)
            nc.sync.dma_start(out=cvt[:], in_=ci_d[:])
            rvt = cpool.tile([128, NB], F32)
            nc.sync.dma_start(out=rvt[:], in_=ri_d[:])
            identw = cpool.tile([128, 262], F32)
            nc.scalar.dma_start(out=identw[:], in_=id_d[:])

            vt_hist = [None] * NB
            vtu_hist = [None] * NB
            in_off = [0]
            out_off = [0]

            def fold_block(k):
                G = BT[k]
                w = G * 2 * D
                gth = gpool.tile([128, w], BF16, tag="gth", name=f"gth{k}")
                nc.sync.dma_start(out=gth[:],
                                  in_=xb_d[:, in_off[0]: in_off[0] + w])
                in_off[0] += w
                r_last = min(6, G - 1)
                vts = []
                for chunk in range(2):
                    vt = vtps.tile([128, RC], F32, tag=f"vt{chunk}",
                                   name=f"vt{chunk}_{k}")
                    # g is split by residue mod 7 so one matmul's out runs
                    # (21m + 3i + c) never self-overlap; the fold overlap
                    # accumulates across matmuls via PSUM has_written.
                    for j in range(7):
                        d = j if chunk == 0 else j - 6
                        for r in range(7):
                            Gr = len(range(r, G, 7))
                            if Gr == 0:
                                continue
                            rhs = _ap(gth[:],
                                      chunk * D + r * 2 * D + j * 21,
                                      [(14 * D, Gr), (3, 7), (1, 3)])
                            out = _ap(vt[:], 3 * r,
                                      [(21, Gr), (3, 7), (1, 3)])
                            nc.tensor.matmul(
                                out, lhsT=identb[:, 128 - d:256 - d],
                                rhs=rhs,
                                start=(j == 0 and r == 0),
                                stop=(j == 6 and r == r_last))
                    vts.append(vt)
                vt_hist[k] = vts
                if k >= 2:
                    vt_hist[k - 2] = None

            def merge_block(k):
                G = BT[k]
                n1 = min(G, 30)              # rows merged from vt_k
                vtus = []
                for chunk in range(2):
                    vt = vt_hist[k][chunk]
                    cv = cvt[:, chunk:chunk + 1]
                    vtu = vtup.tile([128, RC], F32, tag=f"vtu{chunk}",
                                    name=f"vtu{chunk}_{k}")
                    nc.vector.tensor_scalar_mul(
                        out=vtu[:, 36:36 + 3 * n1],
                        in0=vt[:, 18:18 + 3 * n1], scalar1=cv)
                    if k > 0:
                        Bp = BT[k - 1]
                        vtp = vt_hist[k - 1][chunk]
                        nc.vector.tensor_scalar_mul(
                            out=vtu[:, 0:18],
                            in0=vtp[:, 3 * (Bp - 6):3 * (Bp - 6) + 18],
                            scalar1=cv)
                        nc.vector.tensor_copy(
                            out=vtu[:, 18:36],
                            in_=vtp[:, 3 * Bp:3 * Bp + 18])
                        nc.vector.tensor_tensor(
                            out=vtu[:, 18:36], in0=vtu[:, 18:36],
                            in1=vt[:, 0:18], op=mybir.AluOpType.add)
                        nc.vector.tensor_scalar_mul(
                            out=vtu[:, 18:36], in0=vtu[:, 18:36], scalar1=cv)
                    else:
                        nc.vector.tensor_scalar_mul(
                            out=vtu[:, 18:36], in0=vt[:, 0:18], scalar1=cv)
                    vtus.append(vtu)
                vtu_hist[k] = vtus

            def unfold_rest(k):
                vtus = vtu_hist[k]
                vtu_hist[k] = None
                vb = vbp.tile([RC, 256], F32, tag="vb", name=f"vb{k}")
                for chunk in range(2):
                    nc.tensor.matmul(
                        vb[:, chunk * 128:(chunk + 1) * 128],
                        lhsT=vtus[chunk][:, 0:RC], rhs=identw[:, 128:256],
                        is_transpose=True,
                        start=(chunk == 0), stop=(chunk == 1))
                vsb = vsbp.tile([RC, 256], BF16, tag="vsb", name=f"vsb{k}")
                nc.scalar.mul(vsb[:], vb[:], rvt[0:RC, k:k + 1])
                t_lo, t_hi = OT[k]
                nt = t_hi - t_lo + 1
                goff = t_lo - (SB[k] - 6)
                w = nt * 2 * D
                stg = stgp.tile([128, w], BF16, tag="stg", name=f"stg{k}")
                cpy = 0
                for j in range(7):
                    for ci, base in ((0, 0), (1, 122)):
                        pj = pjp.tile([128, RC], BF16, tag="pj",
                                      name=f"pj{k}_{j}_{ci}")
                        nc.tensor.matmul(
                            pj[:], lhsT=vsb[:, base + j: base + j + 128],
                            rhs=identb[0:RC, 128:128 + RC],
                            is_transpose=True, start=True, stop=True)
                        src = _ap(pj[:], goff * 3, [(3, nt), (3, 7), (1, 3)])
                        dst = _ap(stg[:], ci * D + j * 21,
                                  [(2 * D, nt), (3, 7), (1, 3)])
                        if cpy % 2 == 0:
                            nc.scalar.copy(out=dst, in_=src)
                        else:
                            nc.vector.tensor_copy(out=dst, in_=src)
                        cpy += 1
                nc.scalar.dma_start(out=y_d[:, out_off[0]: out_off[0] + w],
                                    in_=stg[:])
                out_off[0] += w

            fold_block(0)
            for k in range(NB):
                merge_block(k)
                if k + 1 < NB:
                    fold_block(k + 1)
                unfold_rest(k)

    nc.compile()
    return nc


_NC_CACHE = [None]


def _build_in_maps(x, nlInds):
    cores = [_host_prep_core(x, nlInds, c) for c in range(8)]
    idw = _identw()
    import ml_dtypes
    idb = idw.astype(ml_dtypes.bfloat16)
    in_maps = [dict(x_bf=cr["x_bf"], colinv=cr["colinv"], rowinv=cr["rowinv"],
                    identw=idw, identb=idb) for cr in cores]
    return cores, in_maps


def kernel(x, nlDists, nlInds, pixels_h, pixels_w):
    x = np.ascontiguousarray(np.asarray(x, dtype=np.float32))
    nlInds = np.asarray(nlInds)
    if _NC_CACHE[0] is None:
        _NC_CACHE[0] = build_nc()
    nc = _NC_CACHE[0]
    cores, in_maps = _build_in_maps(x, nlInds)
    res = run_bass_kernel_spmd(nc, in_maps, list(range(8)))
    out = np.zeros((T, P, 1, 147), np.float32)
    for c in range(8):
        cr = cores[c]
        y = np.asarray(res.results[c]["y_core"]).astype(np.float32)
        ys = y.reshape(128, SOUT, D).transpose(1, 0, 2).reshape(-1, D)
        sidx = cr["sidx"].reshape(-1)
        valid = sidx >= 0
        out[cr["f"], cr["sel"][sidx[valid]], 0] = ys[valid][:, COLPERM]
    return out


# revision 13
# speedup vs baseline: 1.2176x; 1.1236x over previous
"""Trainium2 Bass kernel for nn_Aggregation0 (scatter_memory).

8 cores = 4 frames x 2 image-halves (SPMD, one program). The host pre-sorts
patch rows into destination order per core; patches are stored j-major
(j, i, c) so the fold matmuls stream contiguous 21-element runs.

Device pipeline, software-pipelined as fold(k+1) between merge(k) and the
rest of unfold(k) so PE folds while DVE/ACT stitch and evacuate. Tops are
blocked [9, 36, 36, 33, 12, 5]: a small head block starts compute early
behind the DMA stream, small tail blocks shorten the pipeline drain.
  fold: per (chunk, j, g mod 7) bf16 matmuls vs shifted-identity weights;
    the g-residue split keeps one matmul's out runs non-overlapping while
    the fold overlap accumulates across matmuls in PSUM (has_written).
  merge: DVE stitches the 6-row block overlap, scaled by 1/cnt(col)
  vb: PE transpose to (row,ch)-major; ACT evacuates to SBUF as bf16 with a
    fused per-partition 1/cnt(row) scale (completing the 1/w normalize)
  pj: per (j, left-chunk) bf16 PE transpose back to column-partition layout
  assembly: DVE/ACT strided copies expand rows into patches (bf16)
  -> sequential bf16 store (host inverse-maps).
DMA: input stream + small consts on the SP HWDGE ring, outputs + identw on
the ACT ring so the two streams don't FIFO-serialize.
"""
import sys
if '/opt/trn_rl_repo' not in sys.path:
    sys.path.insert(0, '/opt/trn_rl_repo')
import numpy as np

import concourse.bacc as bacc
import concourse.bass as bass
import concourse.mybir as mybir
import concourse.tile as tile
from concourse.bass_utils import run_bass_kernel_spmd


T, HP, WP = 4, 256, 256
PS = 7
NPOS = 250
P = NPOS * NPOS
BT = [36, 36, 36, 23]        # tops per block
NB = len(BT)
SB = [sum(BT[:k]) for k in range(NB)]     # block start top
RW = 42                      # rows per vt/vtu tile (max B + 6)
RC = 3 * RW                  # 126 (rl, c) columns
GW = 131
NREAL = GW * NPOS            # 32750
F32 = mybir.dt.float32
BF16 = mybir.dt.bfloat16
D = 147
# out-tops per block k: contiguous partition of [0, 130]
OT = [(max(0, SB[k] - 6),
       (SB[k + 1] - 7) if k + 1 < NB else 130) for k in range(NB)]
SIN = sum(BT) * 2            # 262 input slots of [128, 147]
SOUT = sum(t[1] - t[0] + 1 for t in OT) * 2   # 262 output slots
IN_W = SIN * D
OUT_W = SOUT * D

# device patch element order is (j, i, ch); COLPERM maps back to (ch, i, j)
COLPERM = np.zeros(147, np.int64)
PELEM = np.zeros(147, np.int64)
for _c in range(3):
    for _i in range(7):
        for _j in range(7):
            COLPERM[_c * 49 + _i * 7 + _j] = _j * 21 + _i * 3 + _c
            PELEM[_j * 21 + _i * 3 + _c] = _c * 49 + _i * 7 + _j


def _cntf(z):
    z = np.asarray(z, np.float64)
    return np.minimum(6, z) - np.maximum(0, z - 249) + 1


def _host_prep_core(x, nlInds, c):
    f, h = c >> 1, c & 1
    g0 = 0 if h == 0 else 119
    o_lo, o_hi = (0, 124) if h == 0 else (6, 130)
    inds = nlInds[f, :, 0]
    top = inds[:, 1].astype(np.int64)
    left = inds[:, 2].astype(np.int64)
    invperm = np.empty(P, np.int64)
    invperm[top * NPOS + left] = np.arange(P)
    sel = np.nonzero((top >= g0) & (top <= g0 + 130))[0]
    rank = np.full(P, -1, np.int64)
    rank[sel] = np.arange(NREAL)
    ar128 = np.arange(128)
    DM = -1
    gidx = np.zeros((SIN, 128), np.int64)
    s = 0
    for k in range(NB):
        for g in range(BT[k]):
            gt_ = g0 + SB[k] + g
            for ci, base in ((0, 0), (1, 122)):
                gidx[s] = rank[invperm[gt_ * NPOS + base + ar128]]
                s += 1
    sidx = np.full((SOUT, 128), DM, np.int64)
    s = 0
    for k in range(NB):
        t_lo, t_hi = OT[k]
        for lt in range(t_lo, t_hi + 1):
            gt_ = g0 + lt
            for ci, base in ((0, 0), (1, 122)):
                if o_lo <= lt <= o_hi:
                    col = rank[invperm[gt_ * NPOS + base + ar128]]
                    sidx[s] = col
                    if ci == 1:
                        sidx[s, :6] = DM
                s += 1
    # x plane: (128, SIN*147) dest-ordered slots, (j, i, c) element order, bf16
    import ml_dtypes
    xp = np.concatenate([x[f, sel, 0], np.zeros((1, D), np.float32)], axis=0)
    xs = xp[gidx.reshape(-1)][:, PELEM].reshape(SIN, 128, D) \
        .transpose(1, 0, 2).reshape(128, IN_W)
    x_bf = np.ascontiguousarray(xs.astype(ml_dtypes.bfloat16))
    # normalization, factorized: colinv[p, chunk] = 1/cnt(col),
    # rowinv[rl*3+c, k] = 1/cnt(row) (0 for rows outside the image)
    colinv = np.zeros((128, 2), np.float32)
    for chunk in range(2):
        colinv[:, chunk] = 1.0 / _cntf(chunk * 128 + np.arange(128))
    rowinv = np.zeros((128, NB), np.float32)
    for k in range(NB):
        for rl in range(RW):
            gr = g0 + SB[k] - 6 + rl
            lr = SB[k] - 6 + rl
            if 0 <= gr <= 255 and 0 <= lr <= 136:
                rowinv[rl * 3:rl * 3 + 3, k] = 1.0 / _cntf(gr)
    return dict(x_bf=x_bf, colinv=colinv, rowinv=rowinv,
                f=f, sel=sel, sidx=sidx)


def _identw():
    w = np.zeros((128, 262), np.float32)
    w[np.arange(128), np.arange(128) + 128] = 1.0
    return w


def _ap(base, off, dims):
    return bass.AP(base.tensor, base.offset + off,
                   [list(base.ap[0])] + [list(d) for d in dims])


def build_nc():
    nc = bacc.Bacc("TRN2", target_bir_lowering=False, debug=False, num_devices=8)
    xb_d = nc.declare_dram_parameter("x_bf", [128, IN_W], BF16, isOutput=False)
    ib_d = nc.declare_dram_parameter("identb", [128, 262], BF16, isOutput=False)
    ci_d = nc.declare_dram_parameter("colinv", [128, 2], F32, isOutput=False)
    ri_d = nc.declare_dram_parameter("rowinv", [128, NB], F32, isOutput=False)
    id_d = nc.declare_dram_parameter("identw", [128, 262], F32, isOutput=False)
    y_d = nc.declare_dram_parameter("y_core", [128, OUT_W], BF16, isOutput=True)

    with tile.TileContext(nc) as tc:
        with tc.tile_pool(name="const", bufs=1) as cpool, \
             tc.tile_pool(name="gp", bufs=3) as gpool, \
             tc.tile_pool(name="vtp", bufs=2, space="PSUM") as vtps, \
             tc.tile_pool(name="vtu", bufs=2) as vtup, \
             tc.tile_pool(name="vbp", bufs=1, space="PSUM") as vbp, \
             tc.tile_pool(name="vsb", bufs=2) as vsbp, \
             tc.tile_pool(name="pjp", bufs=3, space="PSUM") as pjp, \
             tc.tile_pool(name="stg", bufs=2) as stgp:
            # identb + tiny norm tables lead the SP ring (identb gates the
            # first fold); identw (first needed by vb) and the outputs ride
            # the ACT ring so in/out streams don't serialize.
            identb = cpool.tile([128, 262], BF16)
            nc.sync.dma_start(out=identb[:], in_=ib_d[:])
            cvt = cpool.tile([128, 2], F32)
            nc.sync.dma_start(out=cvt[:], in_=ci_d[:])
            rvt = cpool.tile([128, NB], F32)
            nc.sync.dma_start(out=rvt[:], in_=ri_d[:])
            identw = cpool.tile([128, 262], F32)
            nc.scalar.dma_start(out=identw[:], in_=id_d[:])

            vt_hist = [None] * NB
            vtu_hist = [None] * NB
            in_off = [0]
            out_off = [0]

            def fold_block(k):
                G = BT[k]
                w = G * 2 * D
                gth = gpool.tile([128, w], BF16, tag="gth", name=f"gth{k}")
                nc.sync.dma_start(out=gth[:],
                                  in_=xb_d[:, in_off[0]: in_off[0] + w])
                in_off[0] += w
                r_last = min(6, G - 1)
                vts = []
                for chunk in range(2):
                    vt = vtps.tile([128, RC], F32, tag=f"vt{chunk}",
                                   name=f"vt{chunk}_{k}")
                    # g is split by residue mod 7 so one matmul's out runs
                    # (21m + 3i + c) never self-overlap; the fold overlap
                    # accumulates across matmuls via PSUM has_written.
                    for j in range(7):
                        d = j if chunk == 0 else j - 6
                        for r in range(7):
                            Gr = len(range(r, G, 7))
                            if Gr == 0:
                                continue
                            rhs = _ap(gth[:],
                                      chunk * D + r * 2 * D + j * 21,
                                      [(14 * D, Gr), (3, 7), (1, 3)])
                            out = _ap(vt[:], 3 * r,
                                      [(21, Gr), (3, 7), (1, 3)])
                            nc.tensor.matmul(
                                out, lhsT=identb[:, 128 - d:256 - d],
                                rhs=rhs,
                                start=(j == 0 and r == 0),
                                stop=(j == 6 and r == r_last))
                    vts.append(vt)
                vt_hist[k] = vts
                if k >= 2:
                    vt_hist[k - 2] = None

            def merge_block(k):
                G = BT[k]
                n1 = min(G, 30)              # rows merged from vt_k
                vtus = []
                for chunk in range(2):
                    vt = vt_hist[k][chunk]
                    cv = cvt[:, chunk:chunk + 1]
                    vtu = vtup.tile([128, RC], F32, tag=f"vtu{chunk}",
                                    name=f"vtu{chunk}_{k}")
                    nc.vector.tensor_scalar_mul(
                        out=vtu[:, 36:36 + 3 * n1],
                        in0=vt[:, 18:18 + 3 * n1], scalar1=cv)
                    if k > 0:
                        Bp = BT[k - 1]
                        vtp = vt_hist[k - 1][chunk]
                        nc.vector.tensor_scalar_mul(
                            out=vtu[:, 0:18],
                            in0=vtp[:, 3 * (Bp - 6):3 * (Bp - 6) + 18],
                            scalar1=cv)
                        nc.vector.tensor_copy(
                            out=vtu[:, 18:36],
                            in_=vtp[:, 3 * Bp:3 * Bp + 18])
                        nc.vector.tensor_tensor(
                            out=vtu[:, 18:36], in0=vtu[:, 18:36],
                            in1=vt[:, 0:18], op=mybir.AluOpType.add)
                        nc.vector.tensor_scalar_mul(
                            out=vtu[:, 18:36], in0=vtu[:, 18:36], scalar1=cv)
                    else:
                        nc.vector.tensor_scalar_mul(
                            out=vtu[:, 18:36], in0=vt[:, 0:18], scalar1=cv)
                    vtus.append(vtu)
                vtu_hist[k] = vtus

            def unfold_rest(k):
                vtus = vtu_hist[k]
                vtu_hist[k] = None
                vb = vbp.tile([RC, 256], F32, tag="vb", name=f"vb{k}")
                for chunk in range(2):
                    nc.tensor.matmul(
                        vb[:, chunk * 128:(chunk + 1) * 128],
                        lhsT=vtus[chunk][:, 0:RC], rhs=identw[:, 128:256],
                        is_transpose=True,
                        start=(chunk == 0), stop=(chunk == 1))
                vsb = vsbp.tile([RC, 256], BF16, tag="vsb", name=f"vsb{k}")
                nc.scalar.mul(vsb[:], vb[:], rvt[0:RC, k:k + 1])
                t_lo, t_hi = OT[k]
                nt = t_hi - t_lo + 1
                goff = t_lo - (SB[k] - 6)
                w = nt * 2 * D
                stg = stgp.tile([128, w], BF16, tag="stg", name=f"stg{k}")
                cpy = 0
                for j in range(7):
                    for ci, base in ((0, 0), (1, 122)):
                        pj = pjp.tile([128, RC], BF16, tag="pj",
                                      name=f"pj{k}_{j}_{ci}")
                        nc.tensor.matmul(
                            pj[:], lhsT=vsb[:, base + j: base + j + 128],
                            rhs=identb[0:RC, 128:128 + RC],
                            is_transpose=True, start=True, stop=True)
                        src = _ap(pj[:], goff * 3, [(3, nt), (3, 7), (1, 3)])
                        dst = _ap(stg[:], ci * D + j * 21,
                                  [(2 * D, nt), (3, 7), (1, 3)])
                        # 6 copies on ACT, 8 on DVE (ACT also carries the
                        # vsb evacuation + DMA issue slots)
                        if cpy % 2 == 0 and cpy < 12:
                            nc.scalar.copy(out=dst, in_=src)
                        else:
                            nc.vector.tensor_copy(out=dst, in_=src)
                        cpy += 1
                nc.scalar.dma_start(out=y_d[:, out_off[0]: out_off[0] + w],
                                    in_=stg[:])
                out_off[0] += w

            for k in range(NB):
                fold_block(k)
                merge_block(k)
                unfold_rest(k)

    nc.compile()
    return nc


_NC_CACHE = [None]


def _build_in_maps(x, nlInds):
    cores = [_host_prep_core(x, nlInds, c) for c in range(8)]
    idw = _identw()
    import ml_dtypes
    idb = idw.astype(ml_dtypes.bfloat16)
    in_maps = [dict(x_bf=cr["x_bf"], colinv=cr["colinv"], rowinv=cr["rowinv"],
                    identw=idw, identb=idb) for cr in cores]
    return cores, in_maps


def kernel(x, nlDists, nlInds, pixels_h, pixels_w):
    x = np.ascontiguousarray(np.asarray(x, dtype=np.float32))
    nlInds = np.asarray(nlInds)
    if _NC_CACHE[0] is None:
        _NC_CACHE[0] = build_nc()
    nc = _NC_CACHE[0]
    cores, in_maps = _build_in_maps(x, nlInds)
    res = run_bass_kernel_spmd(nc, in_maps, list(range(8)))
    out = np.zeros((T, P, 1, 147), np.float32)
    for c in range(8):
        cr = cores[c]
        y = np.asarray(res.results[c]["y_core"]).astype(np.float32)
        ys = y.reshape(128, SOUT, D).transpose(1, 0, 2).reshape(-1, D)
        sidx = cr["sidx"].reshape(-1)
        valid = sidx >= 0
        out[cr["f"], cr["sel"][sidx[valid]], 0] = ys[valid][:, COLPERM]
    return out


# revision 15
# speedup vs baseline: 1.2981x; 1.0662x over previous
"""Trainium2 Bass kernel for nn_Aggregation0 (scatter_memory).

8 cores = 4 frames x 2 image-halves (SPMD, one program). The host pre-sorts
patch rows into destination order per core; patches are stored j-major
(j, i, c) so the fold matmuls stream contiguous 21-element runs.

Device pipeline, software-pipelined as fold(k+1) between merge(k) and the
rest of unfold(k) so PE folds while DVE/ACT stitch and evacuate. Tops are
blocked [9, 36, 36, 33, 12, 5]: a small head block starts compute early
behind the DMA stream, small tail blocks shorten the pipeline drain.
  fold: per (chunk, j, g mod 7) bf16 matmuls vs shifted-identity weights;
    the g-residue split keeps one matmul's out runs non-overlapping while
    the fold overlap accumulates across matmuls in PSUM (has_written).
  merge: DVE stitches the 6-row block overlap, scaled by 1/cnt(col)
  vb: PE transpose to (row,ch)-major; ACT evacuates to SBUF as bf16 with a
    fused per-partition 1/cnt(row) scale (completing the 1/w normalize)
  pj: per (j, left-chunk) bf16 PE transpose back to column-partition layout
  assembly: DVE/ACT strided copies expand rows into patches (bf16)
  -> sequential bf16 store (host inverse-maps).
DMA: input stream + small consts on the SP HWDGE ring, outputs + identw on
the ACT ring so the two streams don't FIFO-serialize.
"""
import sys
if '/opt/trn_rl_repo' not in sys.path:
    sys.path.insert(0, '/opt/trn_rl_repo')
import numpy as np

import concourse.bacc as bacc
import concourse.bass as bass
import concourse.mybir as mybir
import concourse.tile as tile
from concourse.bass_utils import run_bass_kernel_spmd


T, HP, WP = 4, 256, 256
PS = 7
NPOS = 250
P = NPOS * NPOS
BT = [36, 36, 36, 23]        # tops per block
NB = len(BT)
SB = [sum(BT[:k]) for k in range(NB)]     # block start top
RW = 42                      # rows per vt/vtu tile (max B + 6)
RC = 3 * RW                  # 126 (rl, c) columns
GW = 131
NREAL = GW * NPOS            # 32750
F32 = mybir.dt.float32
BF16 = mybir.dt.bfloat16
D = 147
# out-tops per block k: contiguous partition of [0, 130]
OT = [(max(0, SB[k] - 6),
       (SB[k + 1] - 7) if k + 1 < NB else 130) for k in range(NB)]
SIN = sum(BT) * 2            # 262 input slots of [128, 147]
SOUT = sum(t[1] - t[0] + 1 for t in OT) * 2   # 262 output slots
IN_W = SIN * D
OUT_W = SOUT * D

# device patch element order is (j, i, ch); COLPERM maps back to (ch, i, j)
COLPERM = np.zeros(147, np.int64)
PELEM = np.zeros(147, np.int64)
for _c in range(3):
    for _i in range(7):
        for _j in range(7):
            COLPERM[_c * 49 + _i * 7 + _j] = _j * 21 + _i * 3 + _c
            PELEM[_j * 21 + _i * 3 + _c] = _c * 49 + _i * 7 + _j


def _cntf(z):
    z = np.asarray(z, np.float64)
    return np.minimum(6, z) - np.maximum(0, z - 249) + 1


def _host_prep_core(x, nlInds, c):
    f, h = c >> 1, c & 1
    g0 = 0 if h == 0 else 119
    o_lo, o_hi = (0, 124) if h == 0 else (6, 130)
    inds = nlInds[f, :, 0]
    top = inds[:, 1].astype(np.int64)
    left = inds[:, 2].astype(np.int64)
    invperm = np.empty(P, np.int64)
    invperm[top * NPOS + left] = np.arange(P)
    sel = np.nonzero((top >= g0) & (top <= g0 + 130))[0]
    rank = np.full(P, -1, np.int64)
    rank[sel] = np.arange(NREAL)
    ar128 = np.arange(128)
    DM = -1
    gidx = np.zeros((SIN, 128), np.int64)
    s = 0
    for k in range(NB):
        for g in range(BT[k]):
            gt_ = g0 + SB[k] + g
            for ci, base in ((0, 0), (1, 122)):
                gidx[s] = rank[invperm[gt_ * NPOS + base + ar128]]
                s += 1
    sidx = np.full((SOUT, 128), DM, np.int64)
    s = 0
    for k in range(NB):
        t_lo, t_hi = OT[k]
        for lt in range(t_lo, t_hi + 1):
            gt_ = g0 + lt
            for ci, base in ((0, 0), (1, 122)):
                if o_lo <= lt <= o_hi:
                    col = rank[invperm[gt_ * NPOS + base + ar128]]
                    sidx[s] = col
                    if ci == 1:
                        sidx[s, :6] = DM
                s += 1
    # x plane: (128, SIN*147) dest-ordered slots, (j, i, c) element order, bf16
    import ml_dtypes
    xp = np.concatenate([x[f, sel, 0], np.zeros((1, D), np.float32)], axis=0)
    xs = xp[gidx.reshape(-1)][:, PELEM].reshape(SIN, 128, D) \
        .transpose(1, 0, 2).reshape(128, IN_W)
    x_bf = np.ascontiguousarray(xs.astype(ml_dtypes.bfloat16))
    # normalization, factorized: colinv[p, chunk] = 1/cnt(col),
    # rowinv[rl*3+c, k] = 1/cnt(row) (0 for rows outside the image)
    colinv = np.zeros((128, 2), np.float32)
    for chunk in range(2):
        colinv[:, chunk] = 1.0 / _cntf(chunk * 128 + np.arange(128))
    rowinv = np.zeros((128, NB), np.float32)
    for k in range(NB):
        for rl in range(RW):
            gr = g0 + SB[k] - 6 + rl
            lr = SB[k] - 6 + rl
            if 0 <= gr <= 255 and 0 <= lr <= 136:
                rowinv[rl * 3:rl * 3 + 3, k] = 1.0 / _cntf(gr)
    return dict(x_bf=x_bf, colinv=colinv, rowinv=rowinv,
                f=f, sel=sel, sidx=sidx)


def _identw():
    w = np.zeros((128, 262), np.float32)
    w[np.arange(128), np.arange(128) + 128] = 1.0
    return w


def _ap(base, off, dims):
    return bass.AP(base.tensor, base.offset + off,
                   [list(base.ap[0])] + [list(d) for d in dims])


def build_nc():
    nc = bacc.Bacc("TRN2", target_bir_lowering=False, debug=False, num_devices=8)
    xb_d = nc.declare_dram_parameter("x_bf", [128, IN_W], BF16, isOutput=False)
    ib_d = nc.declare_dram_parameter("identb", [128, 262], BF16, isOutput=False)
    ci_d = nc.declare_dram_parameter("colinv", [128, 2], F32, isOutput=False)
    ri_d = nc.declare_dram_parameter("rowinv", [128, NB], F32, isOutput=False)
    id_d = nc.declare_dram_parameter("identw", [128, 262], F32, isOutput=False)
    y_d = nc.declare_dram_parameter("y_core", [128, OUT_W], BF16, isOutput=True)

    with tile.TileContext(nc) as tc:
        with tc.tile_pool(name="const", bufs=1) as cpool, \
             tc.tile_pool(name="gp", bufs=3) as gpool, \
             tc.tile_pool(name="vtp", bufs=2, space="PSUM") as vtps, \
             tc.tile_pool(name="vtu", bufs=2) as vtup, \
             tc.tile_pool(name="vbp", bufs=1, space="PSUM") as vbp, \
             tc.tile_pool(name="vsb", bufs=2) as vsbp, \
             tc.tile_pool(name="pjp", bufs=3, space="PSUM") as pjp, \
             tc.tile_pool(name="stg", bufs=2) as stgp:
            # identb + tiny norm tables lead the SP ring (identb gates the
            # first fold); identw (first needed by vb) and the outputs ride
            # the ACT ring so in/out streams don't serialize.
            identb = cpool.tile([128, 262], BF16)
            nc.sync.dma_start(out=identb[:], in_=ib_d[:])
            cvt = cpool.tile([128, 2], F32)
            nc.sync.dma_start(out=cvt[:], in_=ci_d[:])
            rvt = cpool.tile([128, NB], F32)
            nc.sync.dma_start(out=rvt[:], in_=ri_d[:])
            identw = cpool.tile([128, 262], F32)
            nc.scalar.dma_start(out=identw[:], in_=id_d[:])

            vt_hist = [None] * NB
            vtu_hist = [None] * NB
            in_off = [0]
            out_off = [0]

            def fold_block(k):
                G = BT[k]
                w = G * 2 * D
                gth = gpool.tile([128, w], BF16, tag="gth", name=f"gth{k}")
                nc.sync.dma_start(out=gth[:],
                                  in_=xb_d[:, in_off[0]: in_off[0] + w])
                in_off[0] += w
                r_last = min(6, G - 1)
                vts = []
                for chunk in range(2):
                    vt = vtps.tile([128, RC], F32, tag=f"vt{chunk}",
                                   name=f"vt{chunk}_{k}")
                    # g is split by residue mod 7 so one matmul's out runs
                    # (21m + 3i + c) never self-overlap; the fold overlap
                    # accumulates across matmuls via PSUM has_written.
                    for j in range(7):
                        d = j if chunk == 0 else j - 6
                        for r in range(7):
                            Gr = len(range(r, G, 7))
                            if Gr == 0:
                                continue
                            rhs = _ap(gth[:],
                                      chunk * D + r * 2 * D + j * 21,
                                      [(14 * D, Gr), (3, 7), (1, 3)])
                            out = _ap(vt[:], 3 * r,
                                      [(21, Gr), (3, 7), (1, 3)])
                            nc.tensor.matmul(
                                out, lhsT=identb[:, 128 - d:256 - d],
                                rhs=rhs,
                                start=(j == 0 and r == 0),
                                stop=(j == 6 and r == r_last))
                    vts.append(vt)
                vt_hist[k] = vts
                if k >= 2:
                    vt_hist[k - 2] = None

            def merge_block(k):
                G = BT[k]
                n1 = min(G, 30)              # rows merged from vt_k
                vtus = []
                for chunk in range(2):
                    vt = vt_hist[k][chunk]
                    cv = cvt[:, chunk:chunk + 1]
                    vtu = vtup.tile([128, RC], F32, tag=f"vtu{chunk}",
                                    name=f"vtu{chunk}_{k}")
                    nc.vector.tensor_scalar_mul(
                        out=vtu[:, 36:36 + 3 * n1],
                        in0=vt[:, 18:18 + 3 * n1], scalar1=cv)
                    if k > 0:
                        Bp = BT[k - 1]
                        vtp = vt_hist[k - 1][chunk]
                        nc.vector.tensor_scalar_mul(
                            out=vtu[:, 0:18],
                            in0=vtp[:, 3 * (Bp - 6):3 * (Bp - 6) + 18],
                            scalar1=cv)
                        nc.vector.tensor_copy(
                            out=vtu[:, 18:36],
                            in_=vtp[:, 3 * Bp:3 * Bp + 18])
                        nc.vector.tensor_tensor(
                            out=vtu[:, 18:36], in0=vtu[:, 18:36],
                            in1=vt[:, 0:18], op=mybir.AluOpType.add)
                        nc.vector.tensor_scalar_mul(
                            out=vtu[:, 18:36], in0=vtu[:, 18:36], scalar1=cv)
                    else:
                        nc.vector.tensor_scalar_mul(
                            out=vtu[:, 18:36], in0=vt[:, 0:18], scalar1=cv)
                    vtus.append(vtu)
                vtu_hist[k] = vtus

            def unfold_rest(k):
                vtus = vtu_hist[k]
                vtu_hist[k] = None
                vb = vbp.tile([RC, 256], F32, tag="vb", name=f"vb{k}")
                for chunk in range(2):
                    nc.tensor.matmul(
                        vb[:, chunk * 128:(chunk + 1) * 128],
                        lhsT=vtus[chunk][:, 0:RC], rhs=identw[:, 128:256],
                        is_transpose=True,
                        start=(chunk == 0), stop=(chunk == 1))
                vsb = vsbp.tile([RC, 256], BF16, tag="vsb", name=f"vsb{k}")
                nc.scalar.mul(vsb[:], vb[:], rvt[0:RC, k:k + 1])
                t_lo, t_hi = OT[k]
                nt = t_hi - t_lo + 1
                goff = t_lo - (SB[k] - 6)
                w = nt * 2 * D
                # stg is laid out as 14 contiguous (j, ci) slabs of nt*21 so
                # the first 8 slabs can ship while the rest still assemble
                stg = stgp.tile([128, w], BF16, tag="stg", name=f"stg{k}")
                cpy = 0
                for j in range(7):
                    for ci, base in ((0, 0), (1, 122)):
                        pj = pjp.tile([128, RC], BF16, tag="pj",
                                      name=f"pj{k}_{j}_{ci}")
                        nc.tensor.matmul(
                            pj[:], lhsT=vsb[:, base + j: base + j + 128],
                            rhs=identb[0:RC, 128:128 + RC],
                            is_transpose=True, start=True, stop=True)
                        src = _ap(pj[:], goff * 3, [(3, nt), (3, 7), (1, 3)])
                        dst = _ap(stg[:], (j * 2 + ci) * nt * 21,
                                  [(21, nt), (3, 7), (1, 3)])
                        # 5 copies on ACT, 9 on DVE (DVE copies run 2x mode;
                        # ACT also carries vsb evacuation + DMA issue slots)
                        if cpy % 3 == 0:
                            nc.scalar.copy(out=dst, in_=src)
                        else:
                            nc.vector.tensor_copy(out=dst, in_=src)
                        cpy += 1
                        if cpy == 8:
                            wa = 8 * nt * 21
                            nc.scalar.dma_start(
                                out=y_d[:, out_off[0]: out_off[0] + wa],
                                in_=stg[:, 0:wa])
                wa = 8 * nt * 21
                nc.scalar.dma_start(
                    out=y_d[:, out_off[0] + wa: out_off[0] + w],
                    in_=stg[:, wa:w])
                out_off[0] += w

            for k in range(NB):
                fold_block(k)
                merge_block(k)
                unfold_rest(k)

    nc.compile()
    return nc


_NC_CACHE = [None]


def _build_in_maps(x, nlInds):
    cores = [_host_prep_core(x, nlInds, c) for c in range(8)]
    idw = _identw()
    import ml_dtypes
    idb = idw.astype(ml_dtypes.bfloat16)
    in_maps = [dict(x_bf=cr["x_bf"], colinv=cr["colinv"], rowinv=cr["rowinv"],
                    identw=idw, identb=idb) for cr in cores]
    return cores, in_maps


def kernel(x, nlDists, nlInds, pixels_h, pixels_w):
    x = np.ascontiguousarray(np.asarray(x, dtype=np.float32))
    nlInds = np.asarray(nlInds)
    if _NC_CACHE[0] is None:
        _NC_CACHE[0] = build_nc()
    nc = _NC_CACHE[0]
    cores, in_maps = _build_in_maps(x, nlInds)
    res = run_bass_kernel_spmd(nc, in_maps, list(range(8)))
    out = np.zeros((T, P, 1, 147), np.float32)
    for c in range(8):
        cr = cores[c]
        y = np.asarray(res.results[c]["y_core"]).astype(np.float32)
        off = 0
        pieces = []
        for t_lo, t_hi in OT:
            nt = t_hi - t_lo + 1
            seg = y[:, off:off + nt * 2 * D].reshape(128, 7, 2, nt, 21)
            pieces.append(seg.transpose(0, 3, 2, 1, 4).reshape(128, nt * 2, D))
            off += nt * 2 * D
        ys = np.concatenate(pieces, axis=1).transpose(1, 0, 2).reshape(-1, D)
        sidx = cr["sidx"].reshape(-1)
        valid = sidx >= 0
        out[cr["f"], cr["sel"][sidx[valid]], 0] = ys[valid][:, COLPERM]
    return out
